# revision 18
# baseline (speedup 1.0000x reference)
"""Trainium2 Bass kernel for DeTrAttention (dense transformer MHA block).

Full op: out = softmax((q@Wq+bq)(k@Wk+bk)^T / sqrt(64)) (v@Wv+bv) @ Wo + bo
Shapes: q,k,v [B=2, S=2048, H=1024], NH=16 heads, HD=64.

Sharding (8 cores): data-parallel over batch (2 groups of 4 cores); within a
group core r owns query rows [512r, 512r+512) end-to-end AND computes the
K/V projections only for ITS 512 tokens; a single merged byte-packed 4-rank
AllGather (1.05MB -> 4.2MB via DRAM staging) then gives every core the
full-sequence kp/vp.  This removes the 4x-redundant K/V projection compute
(~220k PE cycles/core) of the all-local variant.

Schedule: the Tensor engine only reaches 2.4GHz after ~3us of gap-free
execution (1.2GHz otherwise), and the attention phase alone is exp-paced
(scalar engine), leaving ~1us PE gaps per chunk.  The body is therefore
software-pipelined: iteration i+1's K/V/Q projection matmul groups (and its
AllGather launch) are EMITTED between iteration i's attention chunks, so
the PE queue never drains and the collective latency hides behind a full
attention phase.  kpT/vp/qpT are double-buffered (A/B by iteration parity)
to break the scatter-vs-attention WAR that otherwise serializes the
collective.

Precision: inputs/weights bf16 (Wq,bq pre-scaled by 1/sqrt(64)), fp32 PSUM
accumulation, bf16 scores/probs/values/output-proj (~5e-3 rel err overall).
fp8 attention (DoubleRow) was tested and REJECTED: with near-uniform
attention the context is a mean over ~750 keys (magnitude ~0.04), so fp8's
6% per-element noise does not average away -- measured 2.1e-2 from vp-fp8
alone.  exp(s - 2) guards nothing here in bf16 but costs nothing and keeps
probs in a friendly range; the -2 cancels exactly in the softmax
normalization (ones-column in vp accumulates Z in PSUM row 64).
Biases fold into the PSUM->SBUF copies as per-partition tensor_scalar_adds
(the token-major vp bias arrives pre-broadcast from the host).
"""

import numpy as np

import concourse.bass as bass
import concourse.tile as tile
from concourse import bacc, mybir
from concourse.bass_utils import run_bass_kernel_spmd

F32 = mybir.dt.float32
F32R = mybir.dt.float32r
BF16 = mybir.dt.bfloat16
U8 = mybir.dt.uint8

B, S, H, NH = 2, 2048, 1024, 16
HD = H // NH  # 64
N_CORES = 8
CPG = 4            # cores per batch group
SQ = S // CPG      # query rows per core (512)
SL = S // CPG      # local K/V tokens per core (512)
KB = H // 128      # contraction 128-blocks (8)
MB = H // 128      # output-feature 128-blocks (8)
KTB = S // 128     # key-token 128-blocks (16)
KTL = SL // 128    # local key-token blocks (4)
EBIAS = -2.0       # exp(s + EBIAS); cancels in softmax

KPW = MB * SL * 2              # kp bytes/partition in the gather payload
VW = NH * (HD + 1) * 2         # vp bytes/partition per token-block (2080)
VPW = KTL * VW
AGW = KPW + VPW
GROUPS = [[0, 1, 2, 3], [4, 5, 6, 7]]


def build_nc(sreps=1, upto=3, sim=False):
    """Per-core Bass program (SPMD, identical on all 8 cores).

    sreps > 1 statically unrolls the body for steady-state timing
    (collectives cannot live inside hardware control flow).
    sim=True replaces the AllGather with 4 local DMA broadcasts (same
    DRAM traffic shape) so single-core TimelineSim can model the schedule.
    """
    nc = bacc.Bacc("TRN2", target_bir_lowering=False, debug=False,
                   num_devices=8)

    qT = nc.dram_tensor("qT", [H, SQ], BF16, kind="ExternalInput").ap()
    kT = nc.dram_tensor("kT", [H, SL], BF16, kind="ExternalInput").ap()
    vT = nc.dram_tensor("vT", [H, SL], BF16, kind="ExternalInput").ap()
    Wq = nc.dram_tensor("Wq", [H, H], BF16, kind="ExternalInput").ap()
    Wk = nc.dram_tensor("Wk", [H, H], BF16, kind="ExternalInput").ap()
    Wv = nc.dram_tensor("Wv", [H, H], BF16, kind="ExternalInput").ap()
    Wo = nc.dram_tensor("Wo", [H, H], BF16, kind="ExternalInput").ap()
    bqT = nc.dram_tensor("bqT", [128, MB], F32, kind="ExternalInput").ap()
    bkT = nc.dram_tensor("bkT", [128, MB], F32, kind="ExternalInput").ap()
    boT = nc.dram_tensor("boT", [128, MB], F32, kind="ExternalInput").ap()
    # bv pre-broadcast to all partitions, with a trailing 1.0 column per head
    bvp = nc.dram_tensor("bvp", [128, NH, HD + 1], BF16,
                         kind="ExternalInput").ap()
    outT = nc.dram_tensor("outT", [H, SQ], BF16, kind="ExternalOutput").ap()

    # partition-major views so whole tensors move in one DMA
    qT_p = qT.rearrange("(kb p) t -> p kb t", p=128)
    kT_p = kT.rearrange("(kb p) t -> p kb t", p=128)
    vT_p = vT.rearrange("(kb p) t -> p kb t", p=128)
    Wq_p = Wq.rearrange("(kb p) o -> p kb o", p=128)
    Wk_p = Wk.rearrange("(kb p) o -> p kb o", p=128)
    Wv_p = Wv.rearrange("(kb p) o -> p kb o", p=128)
    Wo_p = Wo.rearrange("(kb p) o -> p kb o", p=128)
    outT_p = outT.rearrange("(ob p) t -> p ob t", p=128)

    with tile.TileContext(nc) as tc:
        with tc.tile_pool(name="consts", bufs=1) as consts, \
             tc.tile_pool(name="persist", bufs=1) as persist, \
             tc.tile_pool(name="stream", bufs=2) as stream, \
             tc.tile_pool(name="wq", bufs=2) as wqp, \
             tc.tile_pool(name="wo", bufs=1) as wop, \
             tc.tile_pool(name="exps", bufs=3) as exps, \
             tc.tile_pool(name="zrp", bufs=1) as zrp, \
             tc.tile_pool(name="stg", bufs=2) as stgp, \
             tc.tile_pool(name="dramp", bufs=2, space="DRAM") as dramp, \
             tc.tile_pool(name="ps2b", bufs=2, space="PSUM") as ps2b, \
             tc.tile_pool(name="psa", bufs=2, space="PSUM") as psA, \
             tc.tile_pool(name="psacc", bufs=2, space="PSUM") as psacc:

            ebias = consts.tile([128, 1], F32)
            nc.vector.memset(ebias, EBIAS)
            bq_sb = consts.tile([128, MB], F32, tag="bq")
            bk_sb = consts.tile([128, MB], F32, tag="bk")
            bo_sb = consts.tile([128, MB], F32, tag="bo")
            bvp_sb = consts.tile([128, NH, HD + 1], BF16, tag="bvp")
            nc.sync.dma_start(out=bq_sb, in_=bqT)
            nc.sync.dma_start(out=bk_sb, in_=bkT)
            nc.sync.dma_start(out=bo_sb, in_=boT)
            nc.sync.dma_start(out=bvp_sb, in_=bvp)

            # double-buffered per-iteration state (parity = iteration % 2)
            kpTs = [persist.tile([128, MB, S], BF16, tag=f"kpT{x}",
                                 name=f"kpT{x}") for x in "AB"]
            vps = [persist.tile([128, KTB, NH, HD + 1], BF16, tag=f"vp{x}",
                                name=f"vp{x}") for x in "AB"]
            qpTs = [persist.tile([128, MB, SQ], BF16, tag=f"qpT{x}",
                                 name=f"qpT{x}") for x in "AB"]
            ctxnTs = [persist.tile([128, MB, SQ], BF16, tag=f"ctxnT{x}",
                                   name=f"ctxnT{x}") for x in "AB"]

            def emit_iter_inputs(it):
                """Emitter closures for iteration `it`'s input pipeline:
                K/V projections staged to DRAM, AllGather launch, Q
                projection.  Each closure is a ~8-matmul unit suitable for
                interleaving into the previous iteration's attention."""
                par = it % 2
                st8 = {}
                ems = []

                def e_kbegin():
                    st8["agi"] = dramp.tile([128, AGW], U8, tag="agi",
                                            name="agi")
                    st8["ago"] = dramp.tile([CPG, 128, AGW], U8, tag="ago",
                                            name="ago")
                    kt_t = stream.tile([128, KB, SL], BF16, tag="in3",
                                       name="kt_t")
                    nc.sync.dma_start(out=kt_t, in_=kT_p)
                    st8["kt"] = kt_t
                    for wh in range(2):
                        w = wqp.tile([128, KB, H // 2], BF16, tag="w",
                                     name=f"wk{wh}")
                        nc.sync.dma_start(
                            out=w, in_=Wk_p[:, :, wh * 512:(wh + 1) * 512])
                        st8[f"wk{wh}"] = w
                ems.append(e_kbegin)

                def e_kp(mb):
                    wh, m = mb // 4, mb % 4
                    ps = psA.tile([128, SL], F32, tag="psa", name=f"kp{mb}")
                    for kb in range(KB):
                        nc.tensor.matmul(
                            ps, st8[f"wk{wh}"][:, kb, m * 128:(m + 1) * 128],
                            st8["kt"][:, kb, :], start=(kb == 0),
                            stop=(kb == KB - 1))
                    kst = stgp.tile([128, SL], BF16, tag="kst", name="kst")
                    nc.vector.tensor_scalar_add(kst, ps, bk_sb[:, mb:mb + 1])
                    nc.sync.dma_start(
                        out=st8["agi"][:, mb * SL * 2:(mb + 1) * SL * 2],
                        in_=kst.bitcast(U8))
                for mb in range(MB):
                    ems.append(lambda mb=mb: e_kp(mb))

                def e_vbegin():
                    vt_t = stream.tile([128, KB, SL], BF16, tag="in3",
                                       name="vt_t")
                    nc.sync.dma_start(out=vt_t, in_=vT_p)
                    st8["vt"] = vt_t
                    for wh in range(2):
                        w = wqp.tile([128, KB, H // 2], BF16, tag="w",
                                     name=f"wv{wh}")
                        nc.sync.dma_start(
                            out=w, in_=Wv_p[:, :, wh * 512:(wh + 1) * 512])
                        st8[f"wv{wh}"] = w
                ems.append(e_vbegin)

                def e_vp(st, wh):
                    # token-major: out [128 tok, 512 ho] ; bias + ones col
                    # fused into the staging copy
                    ps = psA.tile([128, 512], F32, tag="psa",
                                  name=f"vp{st}{wh}")
                    for kb in range(KB):
                        nc.tensor.matmul(
                            ps, st8["vt"][:, kb, st * 128:(st + 1) * 128],
                            st8[f"wv{wh}"][:, kb, :], start=(kb == 0),
                            stop=(kb == KB - 1))
                    hsl = slice(wh * 8, (wh + 1) * 8)
                    vst = stgp.tile([128, 8, HD + 1], BF16, tag="kst",
                                    name="vst")
                    nc.vector.tensor_add(
                        vst[:, :, 0:HD],
                        ps.rearrange("p (hh d) -> p hh d", d=HD),
                        bvp_sb[:, hsl, 0:HD])
                    nc.vector.tensor_copy(vst[:, :, HD:HD + 1],
                                          bvp_sb[:, hsl, HD:HD + 1])
                    off = KPW + st * VW + wh * (VW // 2)
                    nc.sync.dma_start(
                        out=st8["agi"][:, off:off + VW // 2],
                        in_=vst.rearrange("p h c -> p (h c)").bitcast(U8))
                for st in range(KTL):
                    for wh in range(2):
                        ems.append(lambda st=st, wh=wh: e_vp(st, wh))

                def e_ag():
                    if sim:
                        for r in range(CPG):
                            nc.sync.dma_start(out=st8["ago"][r],
                                              in_=st8["agi"])
                    else:
                        nc.gpsimd.collective_compute(
                            "AllGather", mybir.AluOpType.bypass,
                            ins=[st8["agi"].opt()], outs=[st8["ago"].opt()],
                            replica_groups=GROUPS)
                ems.append(e_ag)

                def e_qbegin():
                    qt_t = stream.tile([128, KB, SQ], BF16, tag="in3",
                                       name="qt_t")
                    nc.sync.dma_start(out=qt_t, in_=qT_p)
                    st8["qt"] = qt_t
                    for wh in range(2):
                        w = wqp.tile([128, KB, H // 2], BF16, tag="w",
                                     name=f"wq{wh}")
                        nc.sync.dma_start(
                            out=w, in_=Wq_p[:, :, wh * 512:(wh + 1) * 512])
                        st8[f"wq{wh}"] = w
                ems.append(e_qbegin)

                def e_qp(mb):
                    wh, m = mb // 4, mb % 4
                    ps = psA.tile([128, SQ], F32, tag="psa", name=f"qp{mb}")
                    for kb in range(KB):
                        nc.tensor.matmul(
                            ps, st8[f"wq{wh}"][:, kb, m * 128:(m + 1) * 128],
                            st8["qt"][:, kb, :], start=(kb == 0),
                            stop=(kb == KB - 1))
                    nc.vector.tensor_scalar_add(
                        qpTs[par][:, mb, :], ps, bq_sb[:, mb:mb + 1])
                for mb in range(MB):
                    ems.append(lambda mb=mb: e_qp(mb))

                def e_scatter(r):
                    # kpT/vp[par] are free: attention reads the other parity
                    nc.sync.dma_start(
                        out=kpTs[par][:, :, r * SL:(r + 1) * SL],
                        in_=st8["ago"][r, :, 0:KPW].bitcast(BF16).rearrange(
                            "p (m t) -> p m t", t=SL))
                    nc.sync.dma_start(
                        out=vps[par][:, r * KTL:(r + 1) * KTL],
                        in_=st8["ago"][r, :, KPW:AGW].bitcast(BF16).rearrange(
                            "p (k h c) -> p k h c", h=NH, c=HD + 1))
                for r in range(CPG):
                    ems.append(lambda r=r: e_scatter(r))

                return ems, st8

            def make_outproj(it):
                """Output projection for iteration `it`, as filler units for
                the NEXT iteration's attention (ctxnT is double-buffered)."""
                par = it % 2
                ems = []

                def e_op(ob):
                    wo_t = wop.tile([128, KB, 128], BF16, tag="wo_t")
                    nc.sync.dma_start(
                        out=wo_t, in_=Wo_p[:, :, ob * 128:(ob + 1) * 128])
                    po = psA.tile([128, SQ], F32, tag="psa", name="po")
                    for mb in range(MB):
                        nc.tensor.matmul(po, wo_t[:, mb, :],
                                         ctxnTs[par][:, mb, :],
                                         start=(mb == 0), stop=(mb == MB - 1))
                    ot = stgp.tile([128, SQ], BF16, tag="kst", name="ot")
                    nc.vector.tensor_scalar_add(ot, po, bo_sb[:, ob:ob + 1])
                    nc.sync.dma_start(out=outT_p[:, ob, :], in_=ot)
                for ob in range(MB):
                    ems.append(lambda ob=ob: e_op(ob))
                return ems

            pending, pstate = emit_iter_inputs(0)
            prev_outproj = []
            for it in range(sreps):
                par = it % 2
                kpT, vp, qpT = kpTs[par], vps[par], qpTs[par]
                ctxnT = ctxnTs[par]
                for e in pending:
                    e()

                filler = list(prev_outproj)
                if it + 1 < sreps:
                    nxt, pstate = emit_iter_inputs(it + 1)
                    filler += nxt
                pending = filler  # drained via attention interleave or next it

                if upto < 2:
                    nc.sync.dma_start(out=outT_p[:, 0, :],
                                      in_=kpT[:, 0, 0:SQ])
                    nc.sync.dma_start(out=outT_p[:, 1, :],
                                      in_=qpT[:, 0, :])
                    nc.gpsimd.dma_start(out=outT_p[:, 2, 0:65],
                                        in_=vp[:, 0, 0, :])
                    continue

                # ---- attention; iteration it's output projection and
                # iteration it+1's projections fill the exp-paced PE gaps ----
                n_steps = 2 * 4 * (KTB // 2) * 2
                fstep = len(filler) / n_steps
                facc = 0.0
                fi = 0
                step = 0
                for hf in range(2):
                    for pr in range(4):
                        mb = hf * 4 + pr
                        accs = [psacc.tile([128, SQ], F32, tag="acc",
                                           name=f"acc{j}") for j in range(2)]
                        for cc in range(KTB // 2):
                            for j in range(2):
                                p1 = ps2b.tile([128, 2, SQ], F32, tag="sc",
                                               name=f"sc{j}")
                                for i in range(2):
                                    ktb = cc * 2 + i
                                    nc.tensor.matmul(
                                        p1[:, i, :],
                                        kpT[64 * j:64 * j + 64, mb,
                                            ktb * 128:(ktb + 1) * 128],
                                        qpT[64 * j:64 * j + 64, mb, :],
                                        start=True, stop=True)
                                et = exps.tile([128, 2, SQ], BF16, tag="et",
                                               name=f"et{j}")
                                nc.scalar.activation(
                                    out=et, in_=p1,
                                    func=mybir.ActivationFunctionType.Exp,
                                    bias=ebias[:, 0:1])
                                h = 2 * mb + j
                                for i in range(2):
                                    ktb = cc * 2 + i
                                    nc.tensor.matmul(
                                        accs[j][0:HD + 1, :],
                                        vp[:, ktb, h, :], et[:, i, :],
                                        start=(ktb == 0),
                                        stop=(ktb == KTB - 1))
                                step += 1
                                facc += fstep
                                while fi < len(filler) and facc >= fi + 1:
                                    filler[fi]()
                                    fi += 1
                        for j in range(2):
                            zr = zrp.tile([1, SQ], BF16, tag="zr")
                            with nc.allow_low_precision(
                                    reason="softmax 1/Z; DVE mul"):
                                nc.vector.reciprocal(
                                    zr, accs[j][HD:HD + 1, :])
                            zb = zrp.tile([64, SQ], BF16, tag="zb")
                            nc.gpsimd.partition_broadcast(zb, zr)
                            nc.vector.tensor_mul(
                                ctxnT[64 * j:64 * j + 64, mb, :],
                                accs[j][0:HD, :], zb)
                while fi < len(filler):
                    filler[fi]()
                    fi += 1
                pending = []

                if upto < 3:
                    nc.sync.dma_start(out=outT_p[:, 0, :],
                                      in_=ctxnT[:, 0, :])
                    prev_outproj = []
                    continue

                prev_outproj = make_outproj(it)

            # final iteration's output projection (no later attention)
            for e in prev_outproj:
                e()

    nc.compile()
    return nc


def shard_inputs(q, k, v, Wq, bq, Wk, bk, Wv, bv, Wo, bo):
    """Host-side sharding: per-core input dicts."""
    import ml_dtypes
    bf16 = ml_dtypes.bfloat16
    scale = np.float32(1.0 / np.sqrt(HD))
    c32 = lambda a: np.ascontiguousarray(a, dtype=np.float32)
    cbf = lambda a: np.ascontiguousarray(np.asarray(a, dtype=np.float32),
                                         dtype=bf16)
    Wq_b = cbf(c32(Wq) * scale)
    Wk_b, Wv_b, Wo_b = cbf(Wk), cbf(Wv), cbf(Wo)
    bqT = np.ascontiguousarray((c32(bq) * scale).reshape(MB, 128).T)
    bkT = np.ascontiguousarray(c32(bk).reshape(MB, 128).T)
    boT = np.ascontiguousarray(c32(bo).reshape(MB, 128).T)
    bvp = np.ones((128, NH, HD + 1), dtype=bf16)
    bvp[:, :, 0:HD] = cbf(bv).reshape(1, NH, HD)
    in_maps = []
    for c in range(N_CORES):
        b, r = c // CPG, c % CPG
        sl = slice(r * SQ, (r + 1) * SQ)
        in_maps.append({
            "qT": cbf(q[b, sl, :].T), "kT": cbf(k[b, sl, :].T),
            "vT": cbf(v[b, sl, :].T),
            "Wq": Wq_b, "Wk": Wk_b, "Wv": Wv_b, "Wo": Wo_b,
            "bqT": bqT, "bkT": bkT, "boT": boT, "bvp": bvp,
        })
    return in_maps


_NC_CACHE = {}


def get_nc():
    if "nc" not in _NC_CACHE:
        _NC_CACHE["nc"] = build_nc()
    return _NC_CACHE["nc"]


def kernel(q, k, v, Wq, bq, Wk, bk, Wv, bv, Wo, bo):
    q, k, v = np.asarray(q), np.asarray(k), np.asarray(v)
    in_maps = shard_inputs(q, k, v, Wq, bq, Wk, bk, Wv, bv, Wo, bo)
    nc = get_nc()
    res = run_bass_kernel_spmd(nc, in_maps, core_ids=list(range(N_CORES)))
    out = np.empty((B, S, H), dtype=np.float32)
    for c in range(N_CORES):
        b, r0 = c // CPG, (c % CPG) * SQ
        out[b, r0:r0 + SQ, :] = np.asarray(
            res.results[c]["outT"], dtype=np.float32).T
    return out
